# revision 13
# baseline (speedup 1.0000x reference)
"""Trainium2 Bass kernel for nn_EpiNN_att (dense_transformer).

Math (per batch n, L=512, D=1280, D_hidden=32, 4 heads x head_dim 8):
    first_order[n]  = (x[n] @ w_token) . w_seq + b_seq
    h[n]            = x[n] @ W_proj.T                      # (L, 32)
    S[n]            = (h[n] @ h[n].T) * 1/(4*sqrt(8))      # mean-over-heads QK^T
    second_order[n] = interaction_scale * sum_{l<m} S[n,l,m] * esm[n,l,m]
    out[n]          = first_order[n] + second_order[n]

v4 architecture (per core: NB=8 batches, data-parallel over N across 8 cores):
  * x ships in three per-D-chunk formats, all dithered (error-feedback along
    L) to kill coherent quantization error:
      - fp8e4 chunks: PE consumes directly (f16 weights x fp8 moving operand)
      - u8 chunks: x ~= XSCL*u - XS0; ACT/DVE widen codes to exact-integer
        f16 (split CA_SPLIT cols to ACT, rest to DVE); decode scale folds
        into the weights, offset into the hw-copy bias (from ROUNDED weights
        so the u-mean term cancels exactly)
      - f16 chunks: DMA straight into the matmul tile
  * esm stays host-packed u8 upper-triangle blocks (alpha/255 in grp).
  * Projection g runs as 2-way col-tiled matmuls (tile_position (0,0)/(0,64)):
    two batches stream concurrently, halving PE streaming time.
  * S*esm multiply+reduce fuses into one DVE scalar_tensor_tensor pass
    (accum_out = row sum -> res column); first-order likewise.
  * Final: three tiny chained f32 matmuls collapse res and fo into so [NB,1].
"""

import math

import numpy as np

N, L, D = 64, 512, 1280
DH = 32
N_HEADS, HEAD_DIM = 4, 8
SCALE = 1.0 / (N_HEADS * math.sqrt(HEAD_DIM))
NCORES = 8
NB = N // NCORES  # batches per core
KD = D // 128  # 10 contraction chunks
RL = L // 128  # 4 row chunks
ESEG = [0, 512, 896, 1152]  # packed esm column offsets per row block
EW = 1280  # total packed esm width
XS0 = 4.25  # u8 decode: x = XSCL*u - XS0
XSCL = 2 * XS0 / 255.0

# per-chunk format: '8' fp8e4 direct, 'u' u8+convert, 'f' f16 direct
X_FMT = "88uuuuuuff"
N8 = X_FMT.count("8")
NU = X_FMT.count("u")
NF = X_FMT.count("f")
CA_SPLIT = 2304  # of the NU*L u8 cols, ACT converts [0, CA), DVE the rest

PRECISION = "f16"

_NC_CACHE = {}


def _build(prec="f16", reps=1, mode="full", loop=None):
    key = (prec, reps, mode, loop, X_FMT, CA_SPLIT)
    if key in _NC_CACHE:
        return _NC_CACHE[key]

    import contextlib

    import concourse.mybir as mybir
    import concourse.tile as tile
    from concourse import bacc

    f32 = mybir.dt.float32
    f16 = mybir.dt.float16
    u8 = mybir.dt.uint8
    f8 = mybir.dt.float8e4
    MUL = mybir.AluOpType.mult
    IDENT = mybir.ActivationFunctionType.Identity
    COPY = mybir.ActivationFunctionType.Copy

    nc = bacc.Bacc()

    x8_d = nc.dram_tensor("x8", [NB, 128, N8 * L], f8, kind="ExternalInput")
    xu_d = nc.dram_tensor("xu", [NB, 128, NU * L], u8, kind="ExternalInput")
    xf_d = nc.dram_tensor("xf", [NB, 128, NF * L], f16, kind="ExternalInput")
    esm_d = nc.dram_tensor("esm", [NB, 128, EW], u8, kind="ExternalInput")
    wT_d = nc.dram_tensor("wT", [128, KD, 33], f16, kind="ExternalInput")
    wseq_d = nc.dram_tensor("wseq", [33, L], f16, kind="ExternalInput")
    bias_d = nc.dram_tensor("bias33", [33, 1], f32, kind="ExternalInput")
    grp_d = nc.dram_tensor("grp", [4 * NB, NB], f32, kind="ExternalInput")
    ones_d = nc.dram_tensor("ones", [128, 1], f32, kind="ExternalInput")
    so_d = nc.dram_tensor("so_out", [NB, 1], f32, kind="ExternalOutput")
    live_d = nc.dram_tensor("live_out", [128, 8], f32, kind="ExternalOutput")

    with tile.TileContext(nc) as tc:
        with (
            tc.tile_pool(name="consts", bufs=1) as consts,
            tc.tile_pool(name="x8p", bufs=4) as x8pool,
            tc.tile_pool(name="xup", bufs=4) as xupool,
            tc.tile_pool(name="x16p", bufs=4) as x16pool,
            tc.tile_pool(name="xfp", bufs=4) as xfpool,
            tc.tile_pool(name="epool", bufs=4) as epool,
            tc.tile_pool(name="hwpool", bufs=4) as hwpool,
            tc.tile_pool(name="tpool", bufs=3) as tpool,
            tc.tile_pool(name="respool", bufs=1) as respool,
            tc.tile_pool(name="gpsum", bufs=2, space="PSUM") as gpsum,
            tc.tile_pool(name="spsum", bufs=3, space="PSUM") as spsum,
            tc.tile_pool(name="opsum", bufs=1, space="PSUM") as opsum,
        ):
            wT_sb = consts.tile([128, KD, 33], f16)
            nc.sync.dma_start(out=wT_sb, in_=wT_d[:, :, :])
            wseq_sb = consts.tile([33, L], f16)
            nc.sync.dma_start(out=wseq_sb, in_=wseq_d[:, :])
            bias_sb = consts.tile([33, 1], f32)
            nc.sync.dma_start(out=bias_sb, in_=bias_d[:, :])
            ones33_sb = consts.tile([33, 1], f32)
            nc.sync.dma_start(out=ones33_sb, in_=ones_d[0:33, :])
            grp_sb = consts.tile([4 * NB, NB], f32)
            nc.sync.dma_start(out=grp_sb, in_=grp_d[:, :])
            ones_sb = consts.tile([128, 1], f32)
            nc.sync.dma_start(out=ones_sb, in_=ones_d[:, :])

            if mode in ("pe", "noconv"):
                x8_0 = consts.tile([128, N8 * L], f8, tag="x8_0")
                nc.sync.dma_start(out=x8_0, in_=x8_d[0, :, :])
                xu_0 = consts.tile([128, NU * L], u8, tag="xu_0")
                nc.sync.dma_start(out=xu_0, in_=xu_d[0, :, :])
                xf_0 = consts.tile([128, NF * L], f16, tag="xf_0")
                nc.sync.dma_start(out=xf_0, in_=xf_d[0, :, :])
                et0 = consts.tile([128, EW], u8, tag="et0")
                nc.scalar.dma_start(out=et0, in_=esm_d[0, :, :])
                x16_0 = consts.tile([128, NU * L], f16, tag="x16_0")
                nc.scalar.activation(out=x16_0, in_=xu_0, func=COPY)
            live = consts.tile([128, 8], f32, tag="live")
            nc.vector.memset(live, 0.0)

            loop_cm = tc.For_i(0, loop, 1) if loop else contextlib.nullcontext()
            with loop_cm:
                for rep in range(reps):
                    res = respool.tile([128, 4 * NB], f32, tag="res")
                    fo = respool.tile([33, NB], f32, tag="fo")

                    for pair in range(NB // 2):
                        tiles = []
                        for half in range(2):
                            n = 2 * pair + half
                            if mode == "pe":
                                x8t, xut, xft, et = x8_0, xu_0, xf_0, et0
                            elif mode == "noconv":
                                x8t, xft, et = x8_0, xf_0, et0
                                xut = xu_0
                                nc.sync.dma_start(out=None, in_=None) if False else None
                                tiles.append((n, x8_0, x16_0, xf_0, et0))
                                continue
                            else:
                                x8t = x8pool.tile([128, N8 * L], f8, tag="x8")
                                nc.scalar.dma_start(out=x8t, in_=x8_d[n, :, :])
                                xut = xupool.tile([128, NU * L], u8, tag="xu")
                                nc.sync.dma_start(out=xut, in_=xu_d[n, :, :])
                                xft = xfpool.tile([128, NF * L], f16, tag="xf")
                                nc.scalar.dma_start(out=xft, in_=xf_d[n, :, :])
                                et = epool.tile([128, EW], u8, tag="esm")
                                nc.gpsimd.dma_start(out=et, in_=esm_d[n, :, :])
                            if mode == "dma":
                                scr = tpool.tile([128, 8], f32, tag="scr")
                                nc.scalar.copy(out=scr[:, 0:2], in_=xut[:, 0:2])
                                nc.scalar.copy(out=scr[:, 2:4], in_=et[:, 0:2])
                                nc.scalar.copy(out=scr[:, 4:6], in_=xft[:, 0:2])
                                nc.scalar.copy(
                                    out=scr[:, 6:8], in_=x8t.bitcast(u8)[:, 0:2]
                                )
                                scr2 = tpool.tile([128, 8], f32, tag="scr2")
                                nc.vector.tensor_mul(scr2, scr, scr)
                                nc.vector.tensor_add(live, live, scr2)
                                continue
                            # widen u8 codes to exact-integer f16 (ACT | DVE)
                            x16 = x16pool.tile([128, NU * L], f16, tag="x16")
                            if CA_SPLIT > 0:
                                nc.scalar.activation(
                                    out=x16[:, :CA_SPLIT], in_=xut[:, :CA_SPLIT],
                                    func=COPY,
                                )
                            if CA_SPLIT < NU * L:
                                nc.vector.tensor_copy(
                                    out=x16[:, CA_SPLIT:], in_=xut[:, CA_SPLIT:]
                                )
                            tiles.append((n, x8t, x16, xft, et))

                        if mode == "dma":
                            continue

                        def rhs_for(k, x8t, x16, xft):
                            fmt = X_FMT[k]
                            i = X_FMT[:k].count(fmt)
                            src = {"8": x8t, "u": x16, "f": xft}[fmt]
                            return src[:, i * L : (i + 1) * L]

                        if mode == "notile":
                            g2 = None
                            gsep = []
                            for half, (n, x8t, x16, xft, et) in enumerate(tiles):
                                g1 = gpsum.tile([33, L], f32, tag="g1")
                                for k in range(KD):
                                    nc.tensor.matmul(
                                        g1,
                                        lhsT=wT_sb[:, k, :],
                                        rhs=rhs_for(k, x8t, x16, xft),
                                        start=(k == 0),
                                        stop=(k == KD - 1),
                                    )
                                gsep.append(g1)
                        else:
                            g2 = gpsum.tile([128, L], f32)
                            for k in range(KD):
                                for half, (n, x8t, x16, xft, et) in enumerate(tiles):
                                    nc.tensor.matmul(
                                        g2[64 * half : 64 * half + 33, :],
                                        lhsT=wT_sb[:, k, :],
                                        rhs=rhs_for(k, x8t, x16, xft),
                                        start=(k == 0),
                                        stop=(k == KD - 1),
                                        tile_position=(0, 64 * half),
                                    )

                        for half, (n, x8t, x16, xft, et) in enumerate(tiles):
                            hw = hwpool.tile([33, L], f16)
                            nc.scalar.activation(
                                out=hw,
                                in_=(gsep[half] if mode == "notile"
                                     else g2[64 * half : 64 * half + 33, :]),
                                func=IDENT,
                                bias=bias_sb,
                            )

                            if mode != "nofo":
                                foscr = tpool.tile([33, L], f16, tag="foscr")
                                nc.vector.scalar_tensor_tensor(
                                    out=foscr, in0=hw, scalar=1.0, in1=wseq_sb,
                                    op0=MUL, op1=MUL,
                                    accum_out=fo[:, n : n + 1],
                                )

                            for r in range(RL):
                                rs = 128 * r
                                ncols = L - rs
                                s = spsum.tile([128, L], f32)
                                nc.tensor.matmul(
                                    s[:, :ncols],
                                    lhsT=hw[0:32, rs : rs + 128],
                                    rhs=hw[0:32, rs:L],
                                    start=True, stop=True,
                                )
                                t = tpool.tile([128, L], f16, tag="t")
                                nc_stt = 8 if mode == "nostt" else ncols
                                nc.vector.scalar_tensor_tensor(
                                    out=t[:, :nc_stt], in0=s[:, :nc_stt],
                                    scalar=1.0,
                                    in1=et[:, ESEG[r] : ESEG[r] + nc_stt],
                                    op0=MUL, op1=MUL,
                                    accum_out=res[:, 4 * n + r : 4 * n + r + 1],
                                )

                    if mode == "dma":
                        continue
                    cs = opsum.tile([4 * NB, 1], f32, tag="cs")
                    nc.tensor.matmul(cs, lhsT=res, rhs=ones_sb, start=True, stop=True)
                    cs_sb = respool.tile([4 * NB, 1], f32, tag="cs_sb")
                    nc.scalar.copy(out=cs_sb, in_=cs)

                    so = opsum.tile([NB, 1], f32, tag="so")
                    nc.tensor.matmul(so, lhsT=grp_sb, rhs=cs_sb, start=True, stop=False)
                    if mode == "nofo":
                        nc.tensor.matmul(
                            so, lhsT=grp_sb, rhs=cs_sb, start=False, stop=True
                        )
                    else:
                        nc.tensor.matmul(
                            so, lhsT=fo, rhs=ones33_sb, start=False, stop=True
                        )
                    so_sb = respool.tile([NB, 1], f32, tag="so_sb")
                    nc.scalar.copy(out=so_sb, in_=so)
                    nc.vector.tensor_add(live[0:NB, 0:1], live[0:NB, 0:1], so_sb)
                    if mode != "pe":
                        nc.sync.dma_start(out=so_d[:, :], in_=so_sb)

                if mode in ("pe", "dma"):
                    nc.sync.dma_start(out=so_d[:, :], in_=live[0:NB, 0:1])
                nc.sync.dma_start(out=live_d[:, :], in_=live)

    nc.compile()
    _NC_CACHE[key] = nc
    return nc


def _quant_x(x):
    """Per-format-chunk quantization with error-feedback dithering along L.

    Returns (u8codes (N,L,D_u), fp8vals (N,L,D_8) fp8, f16vals (N,L,D_f))
    where D_* gather the dims of each format's chunks in chunk order.
    """
    import ml_dtypes

    f8dt = ml_dtypes.float8_e4m3
    x = np.asarray(x, np.float32)
    d8 = [d for k in range(KD) if X_FMT[k] == "8" for d in range(128 * k, 128 * k + 128)]
    du = [d for k in range(KD) if X_FMT[k] == "u" for d in range(128 * k, 128 * k + 128)]
    df = [d for k in range(KD) if X_FMT[k] == "f" for d in range(128 * k, 128 * k + 128)]

    xu_part = x[:, :, du]
    u = np.empty(xu_part.shape, np.uint8)
    carry = np.zeros((x.shape[0], len(du)), np.float32)
    for l in range(x.shape[1]):
        v = xu_part[:, l, :] + carry
        q = np.clip(np.round((v + XS0) / XSCL), 0, 255)
        carry = v - (q * XSCL - XS0)
        u[:, l, :] = q.astype(np.uint8)

    x8_part = x[:, :, d8]
    x8 = np.empty(x8_part.shape, f8dt)
    carry = np.zeros((x.shape[0], len(d8)), np.float32)
    for l in range(x.shape[1]):
        v = x8_part[:, l, :] + carry
        q = v.astype(f8dt)
        carry = v - q.astype(np.float32)
        x8[:, l, :] = q

    xf = x[:, :, df].astype(np.float16)
    return u, x8, xf


def _pack_region(a, nchunks):
    """(N, L, nchunks*128) -> (N, 128, nchunks*L) partition-major layout."""
    n, l, dd = a.shape
    assert dd == nchunks * 128
    t = a.transpose(0, 2, 1)  # (N, D, L)
    t = t.reshape(n, nchunks, 128, l).transpose(0, 2, 1, 3)  # (N,128,nc,L)
    return np.ascontiguousarray(t).reshape(n, 128, nchunks * l)


def _prepare(x, esm_priors, w_token, w_seq, b_seq, W_proj, interaction_scale,
             prec=None):
    alpha = SCALE * float(np.asarray(interaction_scale)) / 255.0

    u, x8, xf = _quant_x(x)
    xu_p = _pack_region(u, NU)
    x8_p = _pack_region(x8, N8)
    xf_p = _pack_region(xf, NF)

    e8 = np.round(np.asarray(esm_priors, np.float32) * 255.0).astype(np.uint8)
    dmask = np.triu(np.ones((128, 128), np.uint8), k=1)
    ep = np.zeros((N, 128, EW), np.uint8)
    for r in range(RL):
        rs = 128 * r
        blk = e8[:, rs : rs + 128, rs:L].copy()
        blk[:, :, 0:128] *= dmask[None]
        ep[:, :, ESEG[r] : ESEG[r] + (L - rs)] = blk

    # weights: u-chunks scaled by XSCL; bias from ROUNDED u-chunk weights
    W = np.asarray(W_proj, np.float32)
    wt = np.asarray(w_token, np.float32)
    wTf = np.concatenate([W.T, wt[:, None]], axis=1)  # (D, 33) f32
    scl = np.array([XSCL if X_FMT[k] == "u" else 1.0 for k in range(KD)],
                   np.float32).repeat(128)
    wT16 = (wTf * scl[:, None]).astype(np.float16)
    umask = np.array([X_FMT[k] == "u" for k in range(KD)]).repeat(128)
    bias33 = -(XS0 / XSCL) * wT16[umask].astype(np.float32).sum(0)
    bias33 = bias33[:, None].astype(np.float32)
    wT = np.ascontiguousarray(wT16.reshape(KD, 128, 33).transpose(1, 0, 2))

    wseq = np.zeros((33, L), np.float16)
    wseq[32, :] = np.asarray(w_seq, np.float32).astype(np.float16)
    grp = np.zeros((4 * NB, NB), np.float32)
    for n in range(NB):
        grp[4 * n : 4 * n + 4, n] = alpha
    ones = np.ones((128, 1), np.float32)

    in_maps = []
    for c in range(NCORES):
        sl = slice(c * NB, (c + 1) * NB)
        in_maps.append(
            {
                "x8": x8_p[sl], "xu": xu_p[sl], "xf": xf_p[sl], "esm": ep[sl],
                "wT": wT, "wseq": wseq, "bias33": bias33,
                "grp": grp, "ones": ones,
            }
        )
    return in_maps


def _gather(results, b_seq):
    outs = [r["so_out"].ravel() for r in results]
    return (np.concatenate(outs) + np.float32(np.asarray(b_seq))).astype(np.float32)


def _run(trace=False, prec=None, reps=1, mode="full", **inputs):
    from concourse.bass_utils import run_bass_kernel_spmd

    nc = _build(prec or PRECISION, reps=reps, mode=mode)
    in_maps = _prepare(**inputs)
    res = run_bass_kernel_spmd(nc, in_maps, core_ids=list(range(NCORES)), trace=trace)
    out = _gather(res.results, inputs["b_seq"])
    return out, res


def kernel(**inputs) -> np.ndarray:
    out, _ = _run(trace=False, **inputs)
    return out
